# revision 25
# baseline (speedup 1.0000x reference)
"""Trainium2 Bass kernel for BatchAllTripletWithClustersLossSemiHard (v5).

Math:  loss = S / (C + eps) with, over same-label pairs p=(i,j) and all k,
  z[p,k] = margin + d_ij - d_ik = V_ik - V_ij + 1,  V[i,k] = 2 x_i.x_k - |x_k|^2
  S = sum_p w_j * sum_k relu(z[p,k] masked),  C = #(z>0)   (w>0 lets the
  weight move outside the relu to a per-row postscale).

All V arithmetic runs in bf16 on the PE (full-bf16 V gives ~8e-4 rel err
vs the 2e-2 gate; verified bit-exact on host).  Per 128-pair chunk:
  PE :  z0 = selT @ v_b  (bf16)  +  I @ mt  (fp8e5)      into PSUM
        v_b = -(V+512) bf16;  mt[p,k] = -1 live, +57344 dead, 0 at k=j
  DVE:  e[p] = sum((citer==jcol[p,c])*z0) = z0[p,j] = -(V_ij+512)
        (citer = column-index ramp built once via a broadcast matmul)
  SE :  R = relu(-z0 + e[p]) -> bf16, accum -> sacc[:,c]
        (-z0+e = V_ik-V_ij+1 live, exactly 0 at k=j, <0 dead/pad)
  DVE:  cl = (R > 0) -> bf16   (plain tensor_scalar, 4x mode)
  PE :  cnt_ps[1,:] += ones.T @ cl        (count reduce on the PE)
Phase-grouped emission; inputs staged as few large DMAs ordered so the
V-matmul and the chunk loop start as early as the two HWDGE queues allow.
"""

import numpy as np
import ml_dtypes

import concourse.bass as bass
import concourse.tile as tile
from concourse import bacc, mybir
from concourse.bass_utils import run_bass_kernel_spmd

EPS = 1e-8
BIG = 57344.0  # fp8e5 max normal; |z| stays < 1500 so this always masks
CEN = 512.0
B, D, NCORES = 384, 512, 8
P = 128
NDC = D // P
DT = mybir.dt.float32
BF = mybir.dt.bfloat16
F8 = mybir.dt.float8e5
BF_NP = ml_dtypes.bfloat16
F8_NP = ml_dtypes.float8_e5m2
F16 = mybir.dt.float16


def _host_prep(labels, clusters, weights):
    labels = np.asarray(labels).astype(np.int64)
    clusters = np.asarray(clusters).astype(np.int64)
    weights = np.asarray(weights).astype(np.float32)

    leq = labels[None, :] == labels[:, None]
    rank = np.cumsum(leq.astype(np.int64), axis=1) - 1
    first = leq & (rank % 2 == 1)
    second = leq & (rank % 2 == 0)
    pbase = ~first   # k-mask for same-cluster (i,j) pairs
    qbase = ~second  # k-mask for cross-cluster pairs

    order = np.lexsort((np.arange(B), labels))  # anchors grouped by class
    pairs = [(i, j) for i in order for j in np.where(leq[i])[0] if j != i]
    npairs = len(pairs)
    percore = -(-npairs // NCORES)
    NP = ((percore + P - 1) // P) * P
    NCH = NP // P

    tables = []
    ma_max = 0
    w_max = 0
    for c in range(NCORES):
        cp = pairs[c * percore:(c + 1) * percore]
        anchors = sorted({i for i, _ in cp})
        jset = sorted({j for _, j in cp} | set(anchors))
        ma_max = max(ma_max, len(anchors))
        w_max = max(w_max, len(jset))
        tables.append((cp, anchors, jset))
    MA = ma_max
    W = min(B, ((w_max + 31) // 32) * 32)

    out = []
    for cp, anchors, jset in tables:
        amap = {a: t for t, a in enumerate(anchors)}
        perm = np.array(jset + [k for k in range(B) if k not in set(jset)],
                        dtype=np.int64)
        iperm = np.empty(B, np.int64)
        iperm[perm] = np.arange(B)
        sel = np.zeros((MA, NP), np.float32)
        # mt covers only the window cols [0, W): 0 live, BIG dead, +1 at k=j
        # (the +1 marker rides into e_raw = vg_j + 1, carrying the margin to
        # every column, including the always-live cols >= W that mt omits)
        mt = np.full((NP, W), BIG, np.float32)
        wtab = np.zeros((P, NCH), np.float32)
        jcol = np.full((P, NCH), -1.0, np.float32)
        for p, (i, j) in enumerate(cp):
            sel[amap[i], p] = 1.0
            base = pbase[i] if clusters[i] == clusters[j] else qbase[i]
            roww = np.where(base[perm[:W]], 0.0, np.float32(BIG))
            roww[iperm[i]] = BIG
            roww[iperm[j]] = 1.0  # marker: e_raw = vg_j + 1; z(k=j) exactly 0
            mt[p] = roww
            wtab[p % P, p // P] = weights[labels[j]]
            jcol[p % P, p // P] = float(iperm[j])
        # mt folded for chunked DMA: [P, NCH*W], chunk c at cols [c*W,(c+1)*W)
        mtf = np.ascontiguousarray(
            mt.reshape(NCH, P, W).transpose(1, 0, 2).reshape(P, NCH * W))
        out.append(dict(
            sel=sel.astype(BF_NP),
            mtf=mtf.astype(F8_NP),
            wtab=wtab,
            jcol=jcol,
            perm=perm,
            anchors=np.asarray(anchors, np.int64),
        ))
    return out, NP, MA, W


def _build_program(NP, MA, W):
    NCH = NP // P
    nc = bacc.Bacc("TRN2", target_bir_lowering=False, debug=False,
                   num_devices=NCORES)

    xtf = nc.dram_tensor("xtf", [P, NDC * B], BF, kind="ExternalInput")
    xmyf = nc.dram_tensor("xmyf", [P, NDC * MA], BF, kind="ExternalInput")
    sel = nc.dram_tensor("sel", [MA, NP], BF, kind="ExternalInput")
    mtd = nc.dram_tensor("mtd", [P, NCH * W], F8, kind="ExternalInput")
    wtabd = nc.dram_tensor("wtabd", [P, NCH], DT, kind="ExternalInput")
    jcold = nc.dram_tensor("jcold", [P, NCH], DT, kind="ExternalInput")
    citerd = nc.dram_tensor("citerd", [P, B], F16, kind="ExternalInput")
    identd = nc.dram_tensor("identd", [P, P], F8, kind="ExternalInput")
    outd = nc.dram_tensor("out", [1, 3], DT, kind="ExternalOutput")

    with tile.TileContext(nc) as tc:
        with (
            tc.tile_pool(name="cst", bufs=1) as cst,
            tc.tile_pool(name="sq", bufs=2) as sqp,
            tc.tile_pool(name="jk", bufs=2) as jkp,
            tc.tile_pool(name="rp", bufs=NCH) as rp,
            tc.tile_pool(name="clp", bufs=3) as clp,
            tc.tile_pool(name="vps", bufs=1, space="PSUM") as vpsp,
            tc.tile_pool(name="zps", bufs=4, space="PSUM") as zpsp,
            tc.tile_pool(name="ips", bufs=1, space="PSUM") as ipsp,
            tc.tile_pool(name="cps", bufs=1, space="PSUM") as cpsp,
            tc.tile_pool(name="fps", bufs=1, space="PSUM") as fpsp,
        ):
            # --- input DMAs: xt halves run in parallel on both queues ---
            MT3 = (NCH // 3) * W
            xt_t = cst.tile([P, NDC * B], BF)
            H = NDC * B // 2
            nc.sync.dma_start(xt_t[:, 0:H], xtf[:, 0:H])
            ident_t = cst.tile([P, P], F8)
            nc.sync.dma_start(ident_t[:], identd[:, :])
            mt_all = cst.tile([P, NCH * W], F8)
            nc.sync.dma_start(mt_all[:, 0:MT3], mtd[:, 0:MT3])
            citer = cst.tile([P, B], F16)
            nc.sync.dma_start(citer[:], citerd[:, :])
            nc.sync.dma_start(mt_all[:, 2 * MT3:], mtd[:, 2 * MT3:])

            xmy_t = cst.tile([P, NDC * MA], BF)
            nc.scalar.dma_start(xmy_t[:], xmyf[:, :])
            nc.scalar.dma_start(xt_t[:, H:], xtf[:, H:])
            sel_t = cst.tile([MA, NP], BF)
            nc.scalar.dma_start(sel_t[:], sel[:, :])
            wtab_t = cst.tile([P, NCH], DT)
            nc.scalar.dma_start(wtab_t[:], wtabd[:, :])
            jcol_t = cst.tile([P, NCH], DT)
            nc.scalar.dma_start(jcol_t[:], jcold[:, :])

            # warm the ScalarE activation table set (after the DMA issues so
            # the ~1.5us table load does not delay them; still well before
            # the first real activation)
            dumm = cst.tile([1, 1], DT)
            nc.vector.memset(dumm[:], 0.0)
            dumm2 = cst.tile([1, 1], DT)
            nc.scalar.activation(dumm2[:], dumm[:],
                                 mybir.ActivationFunctionType.Relu)
            nc.scalar.dma_start(mt_all[:, MT3:2 * MT3], mtd[:, MT3:2 * MT3])

            negones = cst.tile([P, MA], BF)
            nc.vector.memset(negones[:], -1.0)
            ones1 = cst.tile([P, 1], DT)
            nc.vector.memset(ones1[:], 1.0)
            onesb = cst.tile([P, 1], BF)
            nc.vector.memset(onesb[:], 1.0)

            # PE warmup: ~4us of dummy matmuls during the DMA wait unthrottles
            # the HAM clock gate (1.2 -> 2.4 GHz) before the real matmuls
            wm_ps = ipsp.tile([P, P], DT)
            for _ in range(24):
                nc.tensor.matmul(wm_ps[0:MA, 0:MA], lhsT=negones[:],
                                 rhs=negones[:], start=True, stop=True)
            # --- V[i,k] = 2 x_i.x_k - |x_k|^2 on PE (all bf16) ---
            v_ps = vpsp.tile([MA, B], DT)
            for dc in range(NDC):
                xs = xt_t[:, dc * B:(dc + 1) * B]
                nc.tensor.matmul(v_ps[:], lhsT=xmy_t[:, dc * MA:(dc + 1) * MA],
                                 rhs=xs, start=(dc == 0), stop=False)
                xsq = sqp.tile([P, B], BF, tag="xsq")
                nc.vector.tensor_tensor(xsq[:], xs, xs,
                                        op=mybir.AluOpType.mult)
                nc.tensor.matmul(v_ps[:], lhsT=negones[:], rhs=xsq[:],
                                 start=False, stop=(dc == NDC - 1))

            # v_b = -(V + 512) in bf16 (centering keeps bf16 ulp ~0.5)
            v_b = cst.tile([MA, B], BF)
            nc.scalar.activation(v_b[:], v_ps[:],
                                 mybir.ActivationFunctionType.Copy,
                                 bias=-CEN, scale=-1.0)

            e_sb = cst.tile([P, NCH], DT)
            sacc = cst.tile([P, NCH], DT)
            cnt_ps = cpsp.tile([1, B], DT)

            # --- phase-grouped chunk pipeline ---
            for c in range(NCH):
                z_ps = zpsp.tile([P, B], DT, tag="z")
                nc.tensor.matmul(z_ps[:], lhsT=sel_t[:, c * P:(c + 1) * P],
                                 rhs=v_b[:], start=True, stop=False)
                nc.tensor.matmul(z_ps[:, 0:W], lhsT=ident_t[:],
                                 rhs=mt_all[:, c * W:(c + 1) * W],
                                 start=False, stop=True,
                                 skip_group_check=True)

                junk = jkp.tile([P, W], BF, tag="junk")
                nc.vector.scalar_tensor_tensor(
                    junk[:], in0=citer[:, 0:W], scalar=jcol_t[:, c:c + 1],
                    in1=z_ps[:, 0:W],
                    op0=mybir.AluOpType.is_equal, op1=mybir.AluOpType.mult,
                    accum_out=e_sb[:, c:c + 1])

                r_t = rp.tile([P, B], BF, tag="r")
                nc.scalar.activation(r_t[:], z_ps[:],
                                     mybir.ActivationFunctionType.Relu,
                                     bias=e_sb[:, c:c + 1], scale=-1.0,
                                     accum_out=sacc[:, c:c + 1])

                cl = clp.tile([P, B], BF, tag="cl")
                nc.vector.tensor_scalar(cl[:], r_t[:], 0.0, None,
                                        op0=mybir.AluOpType.is_gt)
                nc.tensor.matmul(cnt_ps[:], lhsT=onesb[:], rhs=cl[:],
                                 start=(c == 0), stop=(c == NCH - 1))

            # --- final reduction: out = [sum(w*sacc), sum(cacc)] ---
            red = cst.tile([P, 2], DT)
            nc.vector.memset(red[:], 0.0)
            junk2 = cst.tile([P, NCH], DT)
            nc.vector.scalar_tensor_tensor(
                junk2[:], in0=sacc[:], scalar=1.0, in1=wtab_t[:],
                op0=mybir.AluOpType.mult, op1=mybir.AluOpType.mult,
                accum_out=red[:, 0:1])

            f_ps = fpsp.tile([1, 2], DT)
            nc.tensor.matmul(f_ps[:], lhsT=ones1[:], rhs=red[:],
                             start=True, stop=True)
            out_sb = cst.tile([1, 3], DT)
            nc.scalar.copy(out_sb[:, 0:2], f_ps[:])
            nc.vector.tensor_reduce(out_sb[:, 2:3], cnt_ps[:],
                                    mybir.AxisListType.X,
                                    mybir.AluOpType.add)
            nc.sync.dma_start(outd[:, :], out_sb[:])

    nc.compile()
    return nc


def _fold(a, nch):
    # [nch*P_rows, X] -> [P_rows, nch*X] with chunk c at cols [c*X,(c+1)*X)
    n, x = a.shape
    p = n // nch
    return np.ascontiguousarray(
        a.reshape(nch, p, x).transpose(1, 0, 2).reshape(p, nch * x))


def _make_in_maps(embeddings, tables, MA):
    x = np.asarray(embeddings, dtype=np.float32)
    xt = np.ascontiguousarray(x.T)                       # [D, B]
    identity = np.eye(P, dtype=np.float32).astype(F8_NP)
    citer = np.broadcast_to(np.arange(B, dtype=np.float32), (P, B))
    citer = np.ascontiguousarray(citer).astype(np.float16)
    in_maps = []
    for t in tables:
        xtf = _fold(np.ascontiguousarray(xt[:, t["perm"]]), NDC).astype(BF_NP)
        xmy = np.zeros((MA, D), np.float32)
        a = t["anchors"]
        xmy[:len(a)] = 2.0 * x[a]
        xmyf = _fold(np.ascontiguousarray(xmy.T), NDC).astype(BF_NP)
        in_maps.append({
            "identd": identity,
            "citerd": citer,
            "xtf": xtf,
            "xmyf": xmyf,
            "sel": t["sel"],
            "mtd": t["mtf"],
            "wtabd": t["wtab"],
            "jcold": t["jcol"],
        })
    return in_maps


def run(embeddings, labels, clusters, weights, trace=False):
    tables, NP, MA, W = _host_prep(labels, clusters, weights)
    nc = _build_program(NP, MA, W)
    in_maps = _make_in_maps(embeddings, tables, MA)
    res = run_bass_kernel_spmd(nc, in_maps, core_ids=list(range(NCORES)),
                               trace=trace)
    S = 0.0
    C = 0.0
    for r in res.results:
        S += float(r["out"][0, 0])
        C += float(r["out"][0, 1]) + float(r["out"][0, 2])
    loss = np.float32(np.float32(S) / np.float32(C + EPS))
    return np.asarray(loss, dtype=np.float32), res


def kernel(embeddings, labels, clusters, weights):
    loss, _ = run(embeddings, labels, clusters, weights)
    return loss


# revision 28
# speedup vs baseline: 1.0193x; 1.0193x over previous
"""Trainium2 Bass kernel for BatchAllTripletWithClustersLossSemiHard (v5).

Math:  loss = S / (C + eps) with, over same-label pairs p=(i,j) and all k,
  z[p,k] = margin + d_ij - d_ik = V_ik - V_ij + 1,  V[i,k] = 2 x_i.x_k - |x_k|^2
  S = sum_p w_j * sum_k relu(z[p,k] masked),  C = #(z>0)   (w>0 lets the
  weight move outside the relu to a per-row postscale).

All V arithmetic runs in bf16 on the PE (full-bf16 V gives ~8e-4 rel err
vs the 2e-2 gate; verified bit-exact on host).  Per 128-pair chunk:
  PE :  z0 = selT @ v_b  (bf16)  +  I @ mt  (fp8e5)      into PSUM
        v_b = -(V+512) bf16;  mt[p,k] = -1 live, +57344 dead, 0 at k=j
  DVE:  e[p] = sum((citer==jcol[p,c])*z0) = z0[p,j] = -(V_ij+512)
        (citer = column-index ramp built once via a broadcast matmul)
  SE :  R = relu(-z0 + e[p]) -> bf16, accum -> sacc[:,c]
        (-z0+e = V_ik-V_ij+1 live, exactly 0 at k=j, <0 dead/pad)
  DVE:  cl = (R > 0) -> bf16   (plain tensor_scalar, 4x mode)
  PE :  cnt_ps[1,:] += ones.T @ cl        (count reduce on the PE)
Phase-grouped emission; inputs staged as few large DMAs ordered so the
V-matmul and the chunk loop start as early as the two HWDGE queues allow.
"""

import numpy as np
import ml_dtypes

import concourse.bass as bass
import concourse.tile as tile
from concourse import bacc, mybir
from concourse.bass_utils import run_bass_kernel_spmd

EPS = 1e-8
BIG = 57344.0  # fp8e5 max normal; |z| stays < 1500 so this always masks
CEN = 512.0
B, D, NCORES = 384, 512, 8
P = 128
NDC = D // P
DT = mybir.dt.float32
BF = mybir.dt.bfloat16
F8 = mybir.dt.float8e5
BF_NP = ml_dtypes.bfloat16
F8_NP = ml_dtypes.float8_e5m2
F16 = mybir.dt.float16


def _host_prep(labels, clusters, weights):
    labels = np.asarray(labels).astype(np.int64)
    clusters = np.asarray(clusters).astype(np.int64)
    weights = np.asarray(weights).astype(np.float32)

    leq = labels[None, :] == labels[:, None]
    rank = np.cumsum(leq.astype(np.int64), axis=1) - 1
    first = leq & (rank % 2 == 1)
    second = leq & (rank % 2 == 0)
    pbase = ~first   # k-mask for same-cluster (i,j) pairs
    qbase = ~second  # k-mask for cross-cluster pairs

    order = np.lexsort((np.arange(B), labels))  # anchors grouped by class
    pairs = [(i, j) for i in order for j in np.where(leq[i])[0] if j != i]
    npairs = len(pairs)
    percore = -(-npairs // NCORES)
    NP = ((percore + P - 1) // P) * P
    NCH = NP // P

    tables = []
    ma_max = 0
    w_max = 0
    for c in range(NCORES):
        cp = pairs[c * percore:(c + 1) * percore]
        anchors = sorted({i for i, _ in cp})
        jset = sorted({j for _, j in cp} | set(anchors))
        ma_max = max(ma_max, len(anchors))
        w_max = max(w_max, len(jset))
        tables.append((cp, anchors, jset))
    MA = ma_max
    W = min(B, ((w_max + 31) // 32) * 32)

    out = []
    for cp, anchors, jset in tables:
        amap = {a: t for t, a in enumerate(anchors)}
        perm = np.array(jset + [k for k in range(B) if k not in set(jset)],
                        dtype=np.int64)
        iperm = np.empty(B, np.int64)
        iperm[perm] = np.arange(B)
        sel = np.zeros((MA, NP), np.float32)
        # mt covers only the window cols [0, W): 0 live, BIG dead, +1 at k=j
        # (the +1 marker rides into e_raw = vg_j + 1, carrying the margin to
        # every column, including the always-live cols >= W that mt omits)
        mt = np.full((NP, W), BIG, np.float32)
        wtab = np.zeros((P, NCH), np.float32)
        jcol = np.full((P, NCH), -1.0, np.float32)
        for p, (i, j) in enumerate(cp):
            sel[amap[i], p] = 1.0
            base = pbase[i] if clusters[i] == clusters[j] else qbase[i]
            roww = np.where(base[perm[:W]], 0.0, np.float32(BIG))
            roww[iperm[i]] = BIG
            roww[iperm[j]] = 1.0  # marker: e_raw = vg_j + 1; z(k=j) exactly 0
            mt[p] = roww
            wtab[p % P, p // P] = weights[labels[j]]
            jcol[p % P, p // P] = float(iperm[j])
        # mt folded for chunked DMA: [P, NCH*W], chunk c at cols [c*W,(c+1)*W)
        mtf = np.ascontiguousarray(
            mt.reshape(NCH, P, W).transpose(1, 0, 2).reshape(P, NCH * W))
        out.append(dict(
            sel=sel.astype(BF_NP),
            mtf=mtf.astype(F8_NP),
            wtab=wtab,
            jcol=jcol,
            perm=perm,
            anchors=np.asarray(anchors, np.int64),
        ))
    return out, NP, MA, W


def _build_program(NP, MA, W):
    NCH = NP // P
    nc = bacc.Bacc("TRN2", target_bir_lowering=False, debug=False,
                   num_devices=NCORES)

    xtf = nc.dram_tensor("xtf", [P, NDC * B], BF, kind="ExternalInput")
    xmyf = nc.dram_tensor("xmyf", [P, NDC * MA], BF, kind="ExternalInput")
    sel = nc.dram_tensor("sel", [MA, NP], BF, kind="ExternalInput")
    mtd = nc.dram_tensor("mtd", [P, NCH * W], F8, kind="ExternalInput")
    wtabd = nc.dram_tensor("wtabd", [P, NCH], DT, kind="ExternalInput")
    jcold = nc.dram_tensor("jcold", [P, NCH], DT, kind="ExternalInput")
    citerd = nc.dram_tensor("citerd", [P, B], F16, kind="ExternalInput")
    identd = nc.dram_tensor("identd", [P, P], F8, kind="ExternalInput")
    outd = nc.dram_tensor("out", [1, 3], DT, kind="ExternalOutput")

    with tile.TileContext(nc) as tc:
        with (
            tc.tile_pool(name="cst", bufs=1) as cst,
            tc.tile_pool(name="sq", bufs=2) as sqp,
            tc.tile_pool(name="jk", bufs=2) as jkp,
            tc.tile_pool(name="rp", bufs=NCH) as rp,
            tc.tile_pool(name="clp", bufs=3) as clp,
            tc.tile_pool(name="vps", bufs=1, space="PSUM") as vpsp,
            tc.tile_pool(name="zps", bufs=4, space="PSUM") as zpsp,
            tc.tile_pool(name="ips", bufs=1, space="PSUM") as ipsp,
            tc.tile_pool(name="cps", bufs=1, space="PSUM") as cpsp,
            tc.tile_pool(name="fps", bufs=1, space="PSUM") as fpsp,
        ):
            # --- input DMAs: xt halves run in parallel on both queues ---
            MT3 = (NCH // 3) * W
            ident_t = cst.tile([P, P], F8)
            nc.sync.dma_start(ident_t[:], identd[:, :])
            xt_t = cst.tile([P, NDC * B], BF)
            H = NDC * B // 2
            nc.sync.dma_start(xt_t[:, 0:H], xtf[:, 0:H])
            mt_all = cst.tile([P, NCH * W], F8)
            nc.sync.dma_start(mt_all[:, 0:MT3], mtd[:, 0:MT3])
            citer = cst.tile([P, B], F16)
            nc.sync.dma_start(citer[:], citerd[:, :])
            nc.sync.dma_start(mt_all[:, 2 * MT3:], mtd[:, 2 * MT3:])

            xmy_t = cst.tile([P, NDC * MA], BF)
            nc.scalar.dma_start(xmy_t[:], xmyf[:, :])
            nc.scalar.dma_start(xt_t[:, H:], xtf[:, H:])
            sel_t = cst.tile([MA, NP], BF)
            nc.scalar.dma_start(sel_t[:], sel[:, :])
            wtab_t = cst.tile([P, NCH], DT)
            nc.scalar.dma_start(wtab_t[:], wtabd[:, :])
            jcol_t = cst.tile([P, NCH], DT)
            nc.scalar.dma_start(jcol_t[:], jcold[:, :])

            # warm the ScalarE activation table set (after the DMA issues so
            # the ~1.5us table load does not delay them; still well before
            # the first real activation)
            dumm = cst.tile([1, 1], DT)
            nc.vector.memset(dumm[:], 0.0)
            dumm2 = cst.tile([1, 1], DT)
            nc.scalar.activation(dumm2[:], dumm[:],
                                 mybir.ActivationFunctionType.Relu)
            nc.scalar.dma_start(mt_all[:, MT3:2 * MT3], mtd[:, MT3:2 * MT3])

            negones = cst.tile([P, MA], BF)
            nc.vector.memset(negones[:], -1.0)
            ones1 = cst.tile([P, 1], DT)
            nc.vector.memset(ones1[:], 1.0)
            onesb = cst.tile([P, 1], BF)
            nc.vector.memset(onesb[:], 1.0)

            # PE warmup: ~4us of dummy matmuls during the DMA wait unthrottles
            # the HAM clock gate (1.2 -> 2.4 GHz) before the real matmuls
            wm_ps = ipsp.tile([P, P], DT)
            for _ in range(18):
                nc.tensor.matmul(wm_ps[:], lhsT=ident_t[:], rhs=ident_t[:],
                                 start=True, stop=True)
            # --- V[i,k] = 2 x_i.x_k - |x_k|^2 on PE (all bf16) ---
            v_ps = vpsp.tile([MA, B], DT)
            for dc in range(NDC):
                xs = xt_t[:, dc * B:(dc + 1) * B]
                nc.tensor.matmul(v_ps[:], lhsT=xmy_t[:, dc * MA:(dc + 1) * MA],
                                 rhs=xs, start=(dc == 0), stop=False)
                xsq = sqp.tile([P, B], BF, tag="xsq")
                nc.vector.tensor_tensor(xsq[:], xs, xs,
                                        op=mybir.AluOpType.mult)
                nc.tensor.matmul(v_ps[:], lhsT=negones[:], rhs=xsq[:],
                                 start=False, stop=(dc == NDC - 1))

            # v_b = -(V + 512) in bf16 (centering keeps bf16 ulp ~0.5)
            v_b = cst.tile([MA, B], BF)
            nc.scalar.activation(v_b[:], v_ps[:],
                                 mybir.ActivationFunctionType.Copy,
                                 bias=-CEN, scale=-1.0)

            e_sb = cst.tile([P, NCH], DT)
            sacc = cst.tile([P, NCH], DT)
            cnt_ps = cpsp.tile([1, B], DT)

            # --- phase-grouped chunk pipeline ---
            for c in range(NCH):
                z_ps = zpsp.tile([P, B], DT, tag="z")
                nc.tensor.matmul(z_ps[:], lhsT=sel_t[:, c * P:(c + 1) * P],
                                 rhs=v_b[:], start=True, stop=False)
                nc.tensor.matmul(z_ps[:, 0:W], lhsT=ident_t[:],
                                 rhs=mt_all[:, c * W:(c + 1) * W],
                                 start=False, stop=True,
                                 skip_group_check=True)

                junk = jkp.tile([P, W], BF, tag="junk")
                nc.vector.scalar_tensor_tensor(
                    junk[:], in0=citer[:, 0:W], scalar=jcol_t[:, c:c + 1],
                    in1=z_ps[:, 0:W],
                    op0=mybir.AluOpType.is_equal, op1=mybir.AluOpType.mult,
                    accum_out=e_sb[:, c:c + 1])

                r_t = rp.tile([P, B], BF, tag="r")
                nc.scalar.activation(r_t[:], z_ps[:],
                                     mybir.ActivationFunctionType.Relu,
                                     bias=e_sb[:, c:c + 1], scale=-1.0,
                                     accum_out=sacc[:, c:c + 1])

                cl = clp.tile([P, B], BF, tag="cl")
                nc.vector.tensor_scalar(cl[:], r_t[:], 0.0, None,
                                        op0=mybir.AluOpType.is_gt)
                nc.tensor.matmul(cnt_ps[:], lhsT=onesb[:], rhs=cl[:],
                                 start=(c == 0), stop=(c == NCH - 1))

            # --- final reduction: out = [sum(w*sacc), sum(cacc)] ---
            red = cst.tile([P, 2], DT)
            nc.vector.memset(red[:], 0.0)
            junk2 = cst.tile([P, NCH], DT)
            nc.vector.scalar_tensor_tensor(
                junk2[:], in0=sacc[:], scalar=1.0, in1=wtab_t[:],
                op0=mybir.AluOpType.mult, op1=mybir.AluOpType.mult,
                accum_out=red[:, 0:1])

            f_ps = fpsp.tile([1, 2], DT)
            nc.tensor.matmul(f_ps[:], lhsT=ones1[:], rhs=red[:],
                             start=True, stop=True)
            out_sb = cst.tile([1, 3], DT)
            nc.scalar.copy(out_sb[:, 0:2], f_ps[:])
            nc.vector.tensor_reduce(out_sb[:, 2:3], cnt_ps[:],
                                    mybir.AxisListType.X,
                                    mybir.AluOpType.add)
            nc.sync.dma_start(outd[:, :], out_sb[:])

    nc.compile()
    return nc


def _fold(a, nch):
    # [nch*P_rows, X] -> [P_rows, nch*X] with chunk c at cols [c*X,(c+1)*X)
    n, x = a.shape
    p = n // nch
    return np.ascontiguousarray(
        a.reshape(nch, p, x).transpose(1, 0, 2).reshape(p, nch * x))


def _make_in_maps(embeddings, tables, MA):
    x = np.asarray(embeddings, dtype=np.float32)
    xt = np.ascontiguousarray(x.T)                       # [D, B]
    identity = np.eye(P, dtype=np.float32).astype(F8_NP)
    citer = np.broadcast_to(np.arange(B, dtype=np.float32), (P, B))
    citer = np.ascontiguousarray(citer).astype(np.float16)
    in_maps = []
    for t in tables:
        xtf = _fold(np.ascontiguousarray(xt[:, t["perm"]]), NDC).astype(BF_NP)
        xmy = np.zeros((MA, D), np.float32)
        a = t["anchors"]
        xmy[:len(a)] = 2.0 * x[a]
        xmyf = _fold(np.ascontiguousarray(xmy.T), NDC).astype(BF_NP)
        in_maps.append({
            "identd": identity,
            "citerd": citer,
            "xtf": xtf,
            "xmyf": xmyf,
            "sel": t["sel"],
            "mtd": t["mtf"],
            "wtabd": t["wtab"],
            "jcold": t["jcol"],
        })
    return in_maps


def run(embeddings, labels, clusters, weights, trace=False):
    tables, NP, MA, W = _host_prep(labels, clusters, weights)
    nc = _build_program(NP, MA, W)
    in_maps = _make_in_maps(embeddings, tables, MA)
    res = run_bass_kernel_spmd(nc, in_maps, core_ids=list(range(NCORES)),
                               trace=trace)
    S = 0.0
    C = 0.0
    for r in res.results:
        S += float(r["out"][0, 0])
        C += float(r["out"][0, 1]) + float(r["out"][0, 2])
    loss = np.float32(np.float32(S) / np.float32(C + EPS))
    return np.asarray(loss, dtype=np.float32), res


def kernel(embeddings, labels, clusters, weights):
    loss, _ = run(embeddings, labels, clusters, weights)
    return loss
